# revision 1
# baseline (speedup 1.0000x reference)
"""GCN layer kernel for Trainium2 (Bass/Tile), data-parallel over batch.

Reference computation (per batch element):
    deg = A.sum(-1); d = deg ** -0.5
    t   = X @ W.T + b
    out = relu(diag(d) @ A @ diag(d) @ t)

Per-core mapping (8 cores, one batch element each):
  - A streams in as 16 row-tiles [128, 2048] (HWDGE f32 loads), cast
    f32->bf16 on GpSimd (1-input ops run at line rate there), then transposed
    SBUF->SBUF by the DMA xbar (2-byte-only path) into an 8 MB bf16 at_big.
    The tensor engine contracts over partitions, so A's contraction index
    (its column) must live on partitions; the xbar does that off the PE.
    Xbar layout: out[p, 16r + b] = in[r, 128b + p], so the matmul stationary
    for chunk (k-tile b, mu) is a stride-16 AP - no repacking needed.
  - Row degrees reduce on DVE (2x bf16 rate) from the natural bf16 tiles;
    d = sqrt(1/deg) via DVE reciprocal + ACT sqrt.
  - t = X @ W.T in bf16 (X tiles also xbar-transposed; W.T passed
    pre-transposed from host as a layout choice); bias added in f32 from a
    broadcast tile; y = d * t rounded to bf16 by the ACT scale pass.
  - Main matmul accumulates out[mu] = sum_k AT(k,mu).T @ y[k] in PSUM f32:
    8 accumulator banks run during the stream (triangular schedule: product
    (k, mu) is runnable once row-tiles k and mu have both arrived), the
    remaining 8 row-tiles run as a tail batch afterwards.
  - Drain: relu(d * psum) on ACT, then DMA out (f32).
"""

from contextlib import ExitStack

import numpy as np

import concourse.bacc as bacc
import concourse.mybir as mybir
import concourse.tile as tile
from concourse.bass_utils import run_bass_kernel_spmd
from concourse.masks import make_identity

B = 8
N = 2048
F = 256
P = 128
NT = N // P  # 16 row tiles
FT = F // P  # 2 feature tiles
F32 = mybir.dt.float32
BF16 = mybir.dt.bfloat16
COPY = mybir.ActivationFunctionType.Copy
RELU = mybir.ActivationFunctionType.Relu
ACC_SLOTS = 6  # PSUM accumulator banks (2 reserved for transpose staging)


def _emit(ctx: ExitStack, tc: tile.TileContext, A, X, WT, BIAS, OUT):
    nc = tc.nc

    const = ctx.enter_context(tc.tile_pool(name="const", bufs=1))
    stage = ctx.enter_context(tc.tile_pool(name="stage", bufs=4))
    at_pool = ctx.enter_context(tc.tile_pool(name="at", bufs=1))
    outstage = ctx.enter_context(tc.tile_pool(name="outstage", bufs=4))
    psum_acc = ctx.enter_context(
        tc.tile_pool(name="psum_acc", bufs=ACC_SLOTS, space="PSUM")
    )
    psum_tr = ctx.enter_context(tc.tile_pool(name="psum_tr", bufs=2, space="PSUM"))

    ident = const.tile([P, P], BF16, tag="ident")
    make_identity(nc, ident[:, :])
    ident_f32 = const.tile([P, P], F32, tag="identf")
    make_identity(nc, ident_f32[:, :])

    # W.T resident in SBUF as bf16 (f32 HWDGE load + Pool cast)
    wt_stage = const.tile([P, FT * F], F32, tag="wts")
    for phi in range(FT):
        nc.sync.dma_start(
            out=wt_stage[:, phi * F : (phi + 1) * F], in_=WT[phi * P : (phi + 1) * P, :]
        )
    wt_sb = const.tile([P, FT * F], mybir.dt.float32r, tag="wt")
    nc.scalar.copy(wt_sb[:, :], wt_stage[:, :])

    # bias broadcast tile [128, 256] f32 built via ones-column outer product
    b_row = const.tile([1, F], F32, tag="brow")
    nc.sync.dma_start(out=b_row[:, :], in_=BIAS[:, :])
    ones_row = const.tile([1, P], F32, tag="ones")
    nc.vector.memset(ones_row[:, :], 1.0)
    b_psum = psum_acc.tile([P, F], F32, tag="acc", name="b_psum")
    nc.tensor.matmul(b_psum[:, :], ones_row[:, :], b_row[:, :], start=True, stop=True)
    b_bcast = const.tile([P, F], F32, tag="bbc")
    nc.scalar.copy(b_bcast[:, :], b_psum[:, :])

    # degree -> d = sqrt(1/deg) storage, one column per row-tile
    deg = const.tile([P, NT], F32, tag="deg")
    rec = const.tile([P, NT], F32, tag="rec")
    dinv = const.tile([P, NT], F32, tag="dinv")

    # t = X W^T + b in f32; y = bf16 rounded d*t
    t_big = const.tile([P, NT * F], F32, tag="t")
    y_big = const.tile([P, NT * F], BF16, tag="y")

    # transposed adjacency store (xbar 3D-out layout), tile mu at [:, 2048*mu:]:
    # at_big[p, 2048*mu + 128*k + r] = A[128*mu + r, 128*k + p]
    at_big = at_pool.tile([P, NT * N], BF16, tag="at")
    # view [p, mu, k, r]: stationary chunk (k, mu) = at_view[:, mu, k, :] (contiguous)
    at_view = at_big[:, :].rearrange("p (m e r) -> p m e r", m=NT, e=NT)

    # ---- t = X @ W.T + b: one bulk X load, PE f32r transposes in the idle
    # head, mm1 in f32r (2-pass fp32; small). No casts, no xbar. ----
    F32R = mybir.dt.float32r
    xs_f32 = const.tile([P, NT * F], F32, tag="xsf")
    # one DMA: xs_f32[p, 256*mu + f] = X[128*mu + p, f]
    nc.sync.dma_start(
        out=xs_f32[:, :].rearrange("p (m f) -> p m f", m=NT),
        in_=X.rearrange("(m p) f -> p m f", p=P),
    )
    xt_all = const.tile([P, NT * F], F32, tag="xta")
    for mu in range(NT):
        tp = psum_tr.tile([P, 8 * P], BF16, tag="tr", name="xtp")
        tp_f32 = tp[:, : 2 * F].bitcast(F32)  # [128, 256] f32 view of the bank
        for phi in range(FT):
            nc.tensor.transpose(
                tp_f32[:, phi * P : (phi + 1) * P],
                xs_f32[:, (mu * FT + phi) * P : (mu * FT + phi + 1) * P],
                ident_f32[:, :],
            )
        # drain; ACT output rounds to f32r-compatible (f32r mm1 operand)
        nc.scalar.copy(
            xt_all[:, 2 * mu * P : 2 * (mu + 1) * P].bitcast(F32R), tp_f32[:, :]
        )
        t_psum = psum_acc.tile([P, F], F32, tag="acc", name="t_psum")
        for phi in range(FT):
            nc.tensor.matmul(
                t_psum[:, :],
                xt_all[:, (2 * mu + phi) * P : (2 * mu + phi + 1) * P].bitcast(F32R),
                wt_sb[:, phi * F : (phi + 1) * F],
                start=(phi == 0),
                stop=(phi == FT - 1),
            )
        # t + b -> t_big f32 (DVE, PSUM read)
        nc.vector.tensor_add(t_big[:, mu * F : (mu + 1) * F], t_psum[:, :], b_bcast[:, :])

    # ---- stream A row-tiles: degree, d, y, PE transpose, main matmul ----
    PREFETCH = 3
    a_f32_tiles = {}

    def emit_load(j):
        a_f32_tiles[j] = stage.tile([P, N], F32, tag="af", name=f"a_f32_{j}")
        nc.sync.dma_start(out=a_f32_tiles[j][:, :], in_=A[j * P : (j + 1) * P, :])

    for j in range(PREFETCH):
        emit_load(j)

    acc_tiles = {}

    def emit_product(k, mu):
        nc.tensor.matmul(
            acc_tiles[mu][:, :],
            at_view[:, mu, k, :],
            y_big[:, k * F : (k + 1) * F],
            start=(k == 0),
            stop=(k == NT - 1),
        )

    def emit_drain(mu):
        os = outstage.tile([P, F], F32, tag="os")
        nc.scalar.activation(
            os[:, :], acc_tiles[mu][:, :], RELU, scale=dinv[:, mu : mu + 1]
        )
        nc.gpsimd.dma_start(out=OUT[mu * P : (mu + 1) * P, :], in_=os[:, :])

    for i in range(NT):
        if i + PREFETCH < NT:
            emit_load(i + PREFETCH)
        a_f32 = a_f32_tiles.pop(i)
        # one DVE pass: bf16 cast (matmul operand) + row-sum degree accumulator
        a_bf = stage.tile([P, N], BF16, tag="a")
        nc.vector.tensor_scalar(
            out=a_bf[:, :],
            in0=a_f32[:, :],
            scalar1=0.0,
            scalar2=None,
            op0=mybir.AluOpType.add,
            op1=mybir.AluOpType.add,
            accum_out=deg[:, i : i + 1],
        )
        nc.vector.reciprocal(rec[:, i : i + 1], deg[:, i : i + 1])
        nc.scalar.sqrt(dinv[:, i : i + 1], rec[:, i : i + 1])
        # y[i] = d[i] * t[i], rounded to bf16
        nc.scalar.activation(
            y_big[:, i * F : (i + 1) * F],
            t_big[:, i * F : (i + 1) * F],
            COPY,
            scale=dinv[:, i : i + 1],
        )
        # PE transpose-mode (bf16): 8 chunks per PSUM bank, ACT drains to at_big
        for g in range(2):
            tp = psum_tr.tile([P, 8 * P], BF16, tag="tr")
            for j in range(8):
                k = 8 * g + j
                nc.tensor.transpose(
                    tp[:, j * P : (j + 1) * P],
                    a_bf[:, k * P : (k + 1) * P],
                    ident[:, :],
                )
            nc.scalar.copy(
                at_big[:, N * i + 8 * P * g : N * i + 8 * P * (g + 1)], tp[:, :]
            )
        # main-matmul products that just became runnable (early accumulators):
        # every (k, mu) pair with max(k, mu) == i and mu < ACC_SLOTS
        if i < ACC_SLOTS:
            acc_tiles[i] = psum_acc.tile([P, F], F32, tag="acc", name=f"acc_{i}")
            for k in range(i + 1):
                emit_product(k, i)
        for mu in range(min(i, ACC_SLOTS)):
            emit_product(i, mu)

    # ---- drains + tail batches ----
    for mu in range(ACC_SLOTS):
        emit_drain(mu)
    for mu in range(ACC_SLOTS, NT):
        acc_tiles[mu] = psum_acc.tile([P, F], F32, tag="acc", name=f"acc_{mu}")
        for k in range(NT):
            emit_product(k, mu)
        emit_drain(mu)


_cached_nc = None


def _build():
    nc = bacc.Bacc("TRN2", target_bir_lowering=False, debug=False)
    A = nc.dram_tensor("adj", [N, N], F32, kind="ExternalInput").ap()
    X = nc.dram_tensor("x", [N, F], F32, kind="ExternalInput").ap()
    WT = nc.dram_tensor("wt", [F, F], F32, kind="ExternalInput").ap()
    BIAS = nc.dram_tensor("bias", [1, F], F32, kind="ExternalInput").ap()
    OUT = nc.dram_tensor("out", [N, F], F32, kind="ExternalOutput").ap()
    with tile.TileContext(nc) as tc:
        with ExitStack() as ctx:
            _emit(ctx, tc, A, X, WT, BIAS, OUT)
    nc.compile()
    return nc


def get_nc():
    global _cached_nc
    if _cached_nc is None:
        _cached_nc = _build()
    return _cached_nc


def make_in_maps(node_features, adj_matrix, W, b):
    node_features = np.asarray(node_features, dtype=np.float32)
    adj_matrix = np.asarray(adj_matrix, dtype=np.float32)
    wt = np.ascontiguousarray(np.asarray(W, dtype=np.float32).T)
    bias = np.ascontiguousarray(np.asarray(b, dtype=np.float32).reshape(1, F))
    return [
        {
            "adj": np.ascontiguousarray(adj_matrix[c]),
            "x": np.ascontiguousarray(node_features[c]),
            "wt": wt,
            "bias": bias,
        }
        for c in range(B)
    ]


def kernel(node_features, adj_matrix, W, b):
    nc = get_nc()
    in_maps = make_in_maps(node_features, adj_matrix, W, b)
    res = run_bass_kernel_spmd(nc, in_maps, core_ids=list(range(B)))
    return np.stack([r["out"] for r in res.results], axis=0)



# revision 2
# speedup vs baseline: 1.0735x; 1.0735x over previous
"""GCN layer kernel for Trainium2 (Bass/Tile), data-parallel over batch.

Reference computation (per batch element):
    deg = A.sum(-1); d = deg ** -0.5
    t   = X @ W.T + b
    out = relu(diag(d) @ A @ diag(d) @ t)

Per-core mapping (8 cores, one batch element each):
  - A streams in as 16 row-tiles [128, 2048] f32; DVE does a fused
    f32->bf16 cast + row-sum (degree) pass; d = sqrt(1/deg).
  - Tiles are PE-transposed (identity matmul, bf16) through 2 ping-pong
    PSUM banks, drained by ACT into at_big (bf16, xbar-free layout).
  - t = X @ W.T + b computed in bf16 in the head (X cast + PE transpose +
    2-chunk matmul, bias added on DVE from a broadcast tile);
    y[k] = d[k] * t[k] rounded to bf16 by ACT.
  - Main matmul out[mu] = sum_k at(mu,k).T @ y[k] accumulates in PSUM f32.
    12 chains live across the whole stream packed 2-per-bank in 6 banks
    (half-bank sharing: the bank's first matmul uses start=True which
    zeroes the whole 2KB zero-region; the partner chain's first matmul
    uses start=False and overwrites its still-pending half; the bank's
    last matmul carries stop=True). Products are emitted with a 1-step
    lag behind the transposes so the PE never waits on an ACT drain.
    The remaining 4 chains run in the freed transpose banks after the
    stream (~64 products of tail).
  - Drain: relu(d * psum) on ACT, stores batched 4 row-tiles per
    gpsimd (SWDGE) dma_start.
"""

from contextlib import ExitStack

import numpy as np

import concourse.bacc as bacc
import concourse.mybir as mybir
import concourse.tile as tile
from concourse.bass_utils import run_bass_kernel_spmd
from concourse.masks import make_identity

B = 8
N = 2048
F = 256
P = 128
NT = N // P  # 16 row tiles
FT = F // P  # 2 feature chunks
F32 = mybir.dt.float32
BF16 = mybir.dt.bfloat16
COPY = mybir.ActivationFunctionType.Copy
RELU = mybir.ActivationFunctionType.Relu
NCHAIN = 12  # accumulation chains resident for the whole stream (2 per bank)
PREFETCH = 4
STORE_BATCH = 4


def _emit(ctx: ExitStack, tc: tile.TileContext, A, X, WT, BIAS, OUT):
    nc = tc.nc

    const = ctx.enter_context(tc.tile_pool(name="const", bufs=1))
    stage = ctx.enter_context(tc.tile_pool(name="stage", bufs=PREFETCH))
    bfst = ctx.enter_context(tc.tile_pool(name="bfst", bufs=3))
    at_pool = ctx.enter_context(tc.tile_pool(name="at", bufs=1))
    outstage = ctx.enter_context(tc.tile_pool(name="outstage", bufs=2))
    psum_acc = ctx.enter_context(
        tc.tile_pool(name="psum_acc", bufs=NCHAIN // 2, space="PSUM")
    )
    psum_tr = ctx.enter_context(tc.tile_pool(name="psum_tr", bufs=2, space="PSUM"))

    ident = const.tile([P, P], BF16, tag="ident")
    make_identity(nc, ident[:, :])

    # W.T resident in SBUF as bf16
    wt_stage = const.tile([P, FT * F], F32, tag="wts")
    for phi in range(FT):
        nc.sync.dma_start(
            out=wt_stage[:, phi * F : (phi + 1) * F], in_=WT[phi * P : (phi + 1) * P, :]
        )
    wt_sb = const.tile([P, FT * F], BF16, tag="wt")
    nc.vector.tensor_copy(wt_sb[:, :], wt_stage[:, :])

    # bias broadcast tile [128, 256] f32 via ones-column outer product
    b_row = const.tile([1, F], F32, tag="brow")
    nc.sync.dma_start(out=b_row[:, :], in_=BIAS[:, :])
    ones_row = const.tile([1, P], F32, tag="ones")
    nc.vector.memset(ones_row[:, :], 1.0)
    b_psum = psum_tr.tile([P, F], F32, tag="tr", name="b_psum")
    nc.tensor.matmul(b_psum[:, :], ones_row[:, :], b_row[:, :], start=True, stop=True)
    b_bcast = const.tile([P, F], F32, tag="bbc")
    nc.scalar.copy(b_bcast[:, :], b_psum[:, :])

    # degree -> d = sqrt(1/deg), one column per row-tile
    deg = const.tile([P, NT], F32, tag="deg")
    rec = const.tile([P, NT], F32, tag="rec")
    dinv = const.tile([P, NT], F32, tag="dinv")

    t_big = const.tile([P, NT * F], F32, tag="t")
    y_big = const.tile([P, NT * F], BF16, tag="y")

    # transposed adjacency: at_big[p, 2048*mu + 128*k + r] = A[128*mu + r, 128*k + p]
    at_big = at_pool.tile([P, NT * N], BF16, tag="at")
    at_view = at_big[:, :].rearrange("p (m e r) -> p m e r", m=NT, e=NT)

    # ---- X path: load, cast to bf16, PE-transpose, mm1 in bf16 ----
    xs_f32 = const.tile([P, NT * F], F32, tag="xsf")
    nc.sync.dma_start(
        out=xs_f32[:, :].rearrange("p (m f) -> p m f", m=NT),
        in_=X.rearrange("(m p) f -> p m f", p=P),
    )
    # A-tile loads kick off right behind X on the same queue
    a_f32_tiles = {}

    def emit_load(j):
        a_f32_tiles[j] = stage.tile([P, N], F32, tag="af", name=f"a_f32_{j}")
        nc.sync.dma_start(out=a_f32_tiles[j][:, :], in_=A[j * P : (j + 1) * P, :])

    for j in range(PREFETCH):
        emit_load(j)

    xs_bf = const.tile([P, NT * F], BF16, tag="xsb")
    nc.vector.tensor_copy(xs_bf[:, :], xs_f32[:, :])
    xt_all = const.tile([P, NT * FT * P], BF16, tag="xta")

    def emit_tx_group(g):  # transpose chunks (j, phi) for tiles 4g..4g+3
        tpx = psum_tr.tile([P, 8 * P], BF16, tag="tr", name=f"tpx_{g}")
        for u in range(8):
            c = 8 * g + u  # global chunk index = 2*j + phi
            nc.tensor.transpose(
                tpx[:, u * P : (u + 1) * P],
                xs_bf[:, c * P : (c + 1) * P],
                ident[:, :],
            )
        nc.scalar.copy(xt_all[:, g * 8 * P : (g + 1) * 8 * P], tpx[:, :])

    def emit_mm1(j):
        t_psum = psum_tr.tile([P, F], F32, tag="tr", name=f"t_psum_{j}")
        for phi in range(FT):
            nc.tensor.matmul(
                t_psum[:, :],
                xt_all[:, (2 * j + phi) * P : (2 * j + phi + 1) * P],
                wt_sb[:, phi * F : (phi + 1) * F],
                start=(phi == 0),
                stop=(phi == FT - 1),
            )
        nc.vector.tensor_add(t_big[:, j * F : (j + 1) * F], t_psum[:, :], b_bcast[:, :])

    emit_tx_group(0)
    emit_tx_group(1)
    for j in range(4):
        emit_mm1(j)
    emit_tx_group(2)
    for j in range(4, 8):
        emit_mm1(j)
    emit_tx_group(3)
    for j in range(8, 16):
        emit_mm1(j)

    # ---- main accumulation chains ----
    # chains 0..NCHAIN-1 live in psum_acc banks (2 per bank);
    # chains NCHAIN..15 run in the freed transpose banks after the stream.
    acc_banks = [
        psum_acc.tile([P, 2 * F], F32, tag="acc", name=f"accbank_{b_}")
        for b_ in range(NCHAIN // 2)
    ]

    acc_region = {}
    for mu in range(NCHAIN):
        acc_region[mu] = acc_banks[mu // 2][:, (mu % 2) * F : (mu % 2 + 1) * F]

    def emit_product(k, mu):
        first_of_bank = k == 0 and mu % 2 == 0
        last_of_bank = k == NT - 1 and mu % 2 == 1
        nc.tensor.matmul(
            acc_region[mu],
            at_view[:, mu, k, :],
            y_big[:, k * F : (k + 1) * F],
            start=first_of_bank,
            stop=last_of_bank,
        )

    def emit_products_for(m):
        # all products whose max(k, mu) == m, restricted to resident chains
        if m < NCHAIN:
            for k in range(m + 1):
                emit_product(k, m)
        for mu in range(min(m, NCHAIN)):
            emit_product(m, mu)

    # ---- output drain helpers ----
    ostiles = {}

    def emit_drain(mu, region):
        bi = mu // STORE_BATCH
        if bi not in ostiles:
            ostiles[bi] = outstage.tile([P, STORE_BATCH * F], F32, tag="os", name=f"os_{bi}")
        j = mu % STORE_BATCH
        nc.scalar.activation(
            ostiles[bi][:, j * F : (j + 1) * F], region, RELU,
            scale=dinv[:, mu : mu + 1],
        )
        if j == STORE_BATCH - 1:
            lo = bi * STORE_BATCH
            nc.gpsimd.dma_start(
                out=OUT.rearrange("(m p) f -> p m f", p=P)[:, lo : lo + STORE_BATCH, :],
                in_=ostiles[bi][:, :].rearrange("p (m f) -> p m f", m=STORE_BATCH),
            )

    # ---- stream A row-tiles ----
    for i in range(NT):
        if i + PREFETCH < NT:
            emit_load(i + PREFETCH)
        a_f32 = a_f32_tiles.pop(i)
        # fused f32->bf16 cast + row-sum (degree)
        a_bf = bfst.tile([P, N], BF16, tag="a", name=f"a_bf_{i}")
        nc.vector.tensor_scalar(
            out=a_bf[:, :],
            in0=a_f32[:, :],
            scalar1=0.0,
            scalar2=None,
            op0=mybir.AluOpType.add,
            op1=mybir.AluOpType.add,
            accum_out=deg[:, i : i + 1],
        )
        nc.vector.reciprocal(rec[:, i : i + 1], deg[:, i : i + 1])
        nc.scalar.sqrt(dinv[:, i : i + 1], rec[:, i : i + 1])
        # y[i] = d[i] * t[i], rounded to bf16
        nc.scalar.activation(
            y_big[:, i * F : (i + 1) * F],
            t_big[:, i * F : (i + 1) * F],
            COPY,
            scale=dinv[:, i : i + 1],
        )
        # PE transpose of tile i through the 2 ping-pong banks
        for g in range(2):
            tp = psum_tr.tile([P, 8 * P], BF16, tag="tr", name=f"tp_{i}_{g}")
            for u in range(8):
                k = 8 * g + u
                nc.tensor.transpose(
                    tp[:, u * P : (u + 1) * P],
                    a_bf[:, k * P : (k + 1) * P],
                    ident[:, :],
                )
            nc.scalar.copy(
                at_big[:, N * i + 8 * P * g : N * i + 8 * P * (g + 1)], tp[:, :]
            )
        # products lag the transposes by one step so PE never waits on drains
        if i >= 1:
            emit_products_for(i - 1)

    # ---- tail: finish resident chains, run chains 12..15, drain, store ----
    emit_products_for(NT - 1)  # max-index 15 products for resident chains
    for mu in range(0, NCHAIN, 2):
        emit_drain(mu, acc_region[mu])
        emit_drain(mu + 1, acc_region[mu + 1])

    for pair in range(NCHAIN, NT, 2):
        cbank = psum_tr.tile([P, 2 * F], F32, tag="tr", name=f"cbank_{pair}")
        for mu in (pair, pair + 1):
            for k in range(NT):
                nc.tensor.matmul(
                    cbank[:, (mu % 2) * F : (mu % 2 + 1) * F],
                    at_view[:, mu, k, :],
                    y_big[:, k * F : (k + 1) * F],
                    start=(k == 0 and mu == pair),
                    stop=(k == NT - 1 and mu == pair + 1),
                )
        emit_drain(pair, cbank[:, 0:F])
        emit_drain(pair + 1, cbank[:, F : 2 * F])


_cached_nc = None


def _build():
    nc = bacc.Bacc("TRN2", target_bir_lowering=False, debug=False)
    A = nc.dram_tensor("adj", [N, N], F32, kind="ExternalInput").ap()
    X = nc.dram_tensor("x", [N, F], F32, kind="ExternalInput").ap()
    WT = nc.dram_tensor("wt", [F, F], F32, kind="ExternalInput").ap()
    BIAS = nc.dram_tensor("bias", [1, F], F32, kind="ExternalInput").ap()
    OUT = nc.dram_tensor("out", [N, F], F32, kind="ExternalOutput").ap()
    with tile.TileContext(nc) as tc:
        with ExitStack() as ctx:
            _emit(ctx, tc, A, X, WT, BIAS, OUT)
    nc.compile()
    return nc


def get_nc():
    global _cached_nc
    if _cached_nc is None:
        _cached_nc = _build()
    return _cached_nc


def make_in_maps(node_features, adj_matrix, W, b):
    node_features = np.asarray(node_features, dtype=np.float32)
    adj_matrix = np.asarray(adj_matrix, dtype=np.float32)
    wt = np.ascontiguousarray(np.asarray(W, dtype=np.float32).T)
    bias = np.ascontiguousarray(np.asarray(b, dtype=np.float32).reshape(1, F))
    return [
        {
            "adj": np.ascontiguousarray(adj_matrix[c]),
            "x": np.ascontiguousarray(node_features[c]),
            "wt": wt,
            "bias": bias,
        }
        for c in range(B)
    ]


def kernel(node_features, adj_matrix, W, b):
    nc = get_nc()
    in_maps = make_in_maps(node_features, adj_matrix, W, b)
    res = run_bass_kernel_spmd(nc, in_maps, core_ids=list(range(B)))
    return np.stack([r["out"] for r in res.results], axis=0)


# revision 3
# speedup vs baseline: 1.0857x; 1.0114x over previous
"""GCN layer kernel for Trainium2 (Bass/Tile), data-parallel over batch.

Reference computation (per batch element):
    deg = A.sum(-1); d = deg ** -0.5
    t   = X @ W.T + b
    out = relu(diag(d) @ A @ diag(d) @ t)

Per-core mapping (8 cores, one batch element each):
  - A streams in as 16 row-tiles [128, 2048] f32; DVE does a fused
    f32->bf16 cast + row-sum (degree) pass; d = sqrt(1/deg).
  - Tiles are PE-transposed (identity matmul, bf16) through 2 ping-pong
    PSUM banks; drains split between ACT and DVE to balance load.
  - t = X @ W.T + b in bf16, interleaved into the early stream (X cast in
    4 chunks, PE transpose groups, 2-chunk matmuls). The bias is folded
    into each mm1 chain as a K=1 ones x b product that initializes the
    accumulation group, so the drain is a plain ACT copy.
  - Main matmul out[mu] = sum_k at(mu,k).T @ y[k] accumulates in PSUM f32
    with y[k] = d[k] * t[k] (ACT scale pass). 12 chains live across the
    whole stream packed 2-per-bank in 6 banks (half-bank sharing: the
    bank's first matmul uses start=True, which marks the whole 2KB
    zero-region pending-zero; the partner chain's first matmul uses
    start=False and overwrites its still-pending half; the bank's last
    matmul carries stop=True). Products lag the transposes by one step
    so the PE never waits on a drain. Chains 12..15 run in the freed
    transpose banks right after the stream.
  - Drain: relu(d * psum) on ACT, stores batched 4 row-tiles per
    gpsimd (SWDGE) dma_start.
"""

from contextlib import ExitStack

import numpy as np

import concourse.bacc as bacc
import concourse.mybir as mybir
import concourse.tile as tile
from concourse.bass_utils import run_bass_kernel_spmd
from concourse.masks import make_identity

B = 8
N = 2048
F = 256
P = 128
NT = N // P  # 16 row tiles
FT = F // P  # 2 feature chunks
F32 = mybir.dt.float32
BF16 = mybir.dt.bfloat16
COPY = mybir.ActivationFunctionType.Copy
RELU = mybir.ActivationFunctionType.Relu
NCHAIN = 12  # accumulation chains resident for the whole stream (2 per bank)
PREFETCH = 6
STORE_BATCH = 4
DVE_DRAIN_FROM = 6  # from this A-tile on, group-1 transpose drains go to DVE


def _emit(ctx: ExitStack, tc: tile.TileContext, A, X, WT, BIAS, OUT):
    nc = tc.nc

    const = ctx.enter_context(tc.tile_pool(name="const", bufs=1))
    stage = ctx.enter_context(tc.tile_pool(name="stage", bufs=PREFETCH))
    bfst = ctx.enter_context(tc.tile_pool(name="bfst", bufs=3))
    at_pool = ctx.enter_context(tc.tile_pool(name="at", bufs=1))
    outstage = ctx.enter_context(tc.tile_pool(name="outstage", bufs=2))
    psum_acc = ctx.enter_context(
        tc.tile_pool(name="psum_acc", bufs=NCHAIN // 2, space="PSUM")
    )
    psum_tr = ctx.enter_context(tc.tile_pool(name="psum_tr", bufs=2, space="PSUM"))

    ident = const.tile([P, P], BF16, tag="ident")
    make_identity(nc, ident[:, :])

    # ---- DMA dispatch order: wt, bias, A0, X, A1..A5 (prefetch) ----
    wt_stage = const.tile([P, FT * F], F32, tag="wts")
    for phi in range(FT):
        nc.sync.dma_start(
            out=wt_stage[:, phi * F : (phi + 1) * F], in_=WT[phi * P : (phi + 1) * P, :]
        )
    b_row = const.tile([1, F], F32, tag="brow")
    nc.sync.dma_start(out=b_row[:, :], in_=BIAS[:, :])

    a_f32_tiles = {}

    def emit_load(j):
        a_f32_tiles[j] = stage.tile([P, N], F32, tag="af", name=f"a_f32_{j}")
        nc.sync.dma_start(out=a_f32_tiles[j][:, :], in_=A[j * P : (j + 1) * P, :])

    emit_load(0)
    xs_f32 = const.tile([P, NT * F], F32, tag="xsf")
    nc.sync.dma_start(
        out=xs_f32[:, :].rearrange("p (m f) -> p m f", m=NT),
        in_=X.rearrange("(m p) f -> p m f", p=P),
    )
    for j in range(1, PREFETCH):
        emit_load(j)

    # small bf16 constants (DVE, early)
    wt_sb = const.tile([P, FT * F], BF16, tag="wt")
    nc.vector.tensor_copy(wt_sb[:, :], wt_stage[:, :])
    b_bf = const.tile([1, F], BF16, tag="bbf")
    nc.vector.tensor_copy(b_bf[:, :], b_row[:, :])
    ones_bf = const.tile([1, P], BF16, tag="ones")
    nc.vector.memset(ones_bf[:, :], 1.0)

    deg = const.tile([P, NT], F32, tag="deg")
    rec = const.tile([P, NT], F32, tag="rec")
    dinv = const.tile([P, NT], F32, tag="dinv")

    t_big = const.tile([P, NT * F], F32, tag="t")
    y_big = const.tile([P, NT * F], BF16, tag="y")
    xs_bf = const.tile([P, NT * F], BF16, tag="xsb")
    xt_all = const.tile([P, NT * FT * P], BF16, tag="xta")

    # transposed adjacency: at_big[p, 2048*mu + 128*k + r] = A[128*mu + r, 128*k + p]
    at_big = at_pool.tile([P, NT * N], BF16, tag="at")
    at_view = at_big[:, :].rearrange("p (m e r) -> p m e r", m=NT, e=NT)

    # ---- X-path helpers (interleaved into early stream steps) ----
    def emit_x_chunk(g):  # cast chunk g, transpose group g, mm1 for tiles 4g..4g+3
        nc.vector.tensor_copy(
            xs_bf[:, g * 8 * P : (g + 1) * 8 * P], xs_f32[:, g * 8 * P : (g + 1) * 8 * P]
        )
        tpx = psum_tr.tile([P, 8 * P], BF16, tag="tr", name=f"tpx_{g}")
        for u in range(8):
            c = 8 * g + u
            nc.tensor.transpose(
                tpx[:, u * P : (u + 1) * P], xs_bf[:, c * P : (c + 1) * P], ident[:, :]
            )
        nc.scalar.copy(xt_all[:, g * 8 * P : (g + 1) * 8 * P], tpx[:, :])
        for j in range(4 * g, 4 * g + 4):
            t_psum = psum_tr.tile([P, F], F32, tag="tr", name=f"t_psum_{j}")
            nc.tensor.matmul(
                t_psum[:, :], ones_bf[:, :], b_bf[:, :], start=True, stop=False
            )
            for phi in range(FT):
                nc.tensor.matmul(
                    t_psum[:, :],
                    xt_all[:, (2 * j + phi) * P : (2 * j + phi + 1) * P],
                    wt_sb[:, phi * F : (phi + 1) * F],
                    start=False,
                    stop=(phi == FT - 1),
                )
            nc.scalar.copy(t_big[:, j * F : (j + 1) * F], t_psum[:, :])

    # ---- main accumulation chains (2 per bank, half-bank sharing) ----
    acc_banks = [
        psum_acc.tile([P, 2 * F], F32, tag="acc", name=f"accbank_{b_}")
        for b_ in range(NCHAIN // 2)
    ]
    acc_region = {
        mu: acc_banks[mu // 2][:, (mu % 2) * F : (mu % 2 + 1) * F]
        for mu in range(NCHAIN)
    }

    def emit_product(k, mu):
        nc.tensor.matmul(
            acc_region[mu],
            at_view[:, mu, k, :],
            y_big[:, k * F : (k + 1) * F],
            start=(k == 0 and mu % 2 == 0),
            stop=(k == NT - 1 and mu % 2 == 1),
        )

    def emit_products_for(m):
        if m < NCHAIN:
            for k in range(m + 1):
                emit_product(k, m)
        for mu in range(min(m, NCHAIN)):
            emit_product(m, mu)

    ostiles = {}

    def emit_drain(mu, region):
        bi = mu // STORE_BATCH
        if bi not in ostiles:
            ostiles[bi] = outstage.tile(
                [P, STORE_BATCH * F], F32, tag="os", name=f"os_{bi}"
            )
        j = mu % STORE_BATCH
        nc.scalar.activation(
            ostiles[bi][:, j * F : (j + 1) * F], region, RELU,
            scale=dinv[:, mu : mu + 1],
        )
        if j == STORE_BATCH - 1:
            lo = bi * STORE_BATCH
            nc.gpsimd.dma_start(
                out=OUT.rearrange("(m p) f -> p m f", p=P)[:, lo : lo + STORE_BATCH, :],
                in_=ostiles[bi][:, :].rearrange("p (m f) -> p m f", m=STORE_BATCH),
            )

    # ---- stream A row-tiles ----
    for i in range(NT):
        if i + PREFETCH < NT:
            emit_load(i + PREFETCH)
        a_f32 = a_f32_tiles.pop(i)
        # fused f32->bf16 cast + row-sum (degree)
        a_bf = bfst.tile([P, N], BF16, tag="a", name=f"a_bf_{i}")
        nc.vector.tensor_scalar(
            out=a_bf[:, :],
            in0=a_f32[:, :],
            scalar1=0.0,
            scalar2=None,
            op0=mybir.AluOpType.add,
            op1=mybir.AluOpType.add,
            accum_out=deg[:, i : i + 1],
        )
        nc.vector.reciprocal(rec[:, i : i + 1], deg[:, i : i + 1])

        if 1 <= i <= 4:
            emit_x_chunk(i - 1)

        # PE transpose of tile i through the 2 ping-pong banks
        for g in range(2):
            tp = psum_tr.tile([P, 8 * P], BF16, tag="tr", name=f"tp_{i}_{g}")
            for u in range(8):
                k = 8 * g + u
                nc.tensor.transpose(
                    tp[:, u * P : (u + 1) * P], a_bf[:, k * P : (k + 1) * P], ident[:, :]
                )
            dst = at_big[:, N * i + 8 * P * g : N * i + 8 * P * (g + 1)]
            if g == 1 and i >= DVE_DRAIN_FROM:
                nc.vector.tensor_copy(dst, tp[:, :])
            else:
                nc.scalar.copy(dst, tp[:, :])

        nc.scalar.sqrt(dinv[:, i : i + 1], rec[:, i : i + 1])
        if i == 1:
            nc.scalar.activation(
                y_big[:, 0:F], t_big[:, 0:F], COPY, scale=dinv[:, 0:1]
            )
        if i >= 1:
            nc.scalar.activation(
                y_big[:, i * F : (i + 1) * F],
                t_big[:, i * F : (i + 1) * F],
                COPY,
                scale=dinv[:, i : i + 1],
            )
            emit_products_for(i - 1)

    # ---- tail: finish resident chains, run chains 12..15, drain, store ----
    m = NT - 1
    for pair in range(0, NCHAIN, 2):
        emit_product(m, pair)
        emit_product(m, pair + 1)
        emit_drain(pair, acc_region[pair])
        emit_drain(pair + 1, acc_region[pair + 1])

    for pair in range(NCHAIN, NT, 2):
        cbank = psum_tr.tile([P, 2 * F], F32, tag="tr", name=f"cbank_{pair}")
        for mu in (pair, pair + 1):
            for k in range(NT):
                nc.tensor.matmul(
                    cbank[:, (mu % 2) * F : (mu % 2 + 1) * F],
                    at_view[:, mu, k, :],
                    y_big[:, k * F : (k + 1) * F],
                    start=(k == 0 and mu == pair),
                    stop=(k == NT - 1 and mu == pair + 1),
                )
        emit_drain(pair, cbank[:, 0:F])
        emit_drain(pair + 1, cbank[:, F : 2 * F])


_cached_nc = None


def _build():
    nc = bacc.Bacc("TRN2", target_bir_lowering=False, debug=False)
    A = nc.dram_tensor("adj", [N, N], F32, kind="ExternalInput").ap()
    X = nc.dram_tensor("x", [N, F], F32, kind="ExternalInput").ap()
    WT = nc.dram_tensor("wt", [F, F], F32, kind="ExternalInput").ap()
    BIAS = nc.dram_tensor("bias", [1, F], F32, kind="ExternalInput").ap()
    OUT = nc.dram_tensor("out", [N, F], F32, kind="ExternalOutput").ap()
    with tile.TileContext(nc) as tc:
        with ExitStack() as ctx:
            _emit(ctx, tc, A, X, WT, BIAS, OUT)
    nc.compile()
    return nc


def get_nc():
    global _cached_nc
    if _cached_nc is None:
        _cached_nc = _build()
    return _cached_nc


def make_in_maps(node_features, adj_matrix, W, b):
    node_features = np.asarray(node_features, dtype=np.float32)
    adj_matrix = np.asarray(adj_matrix, dtype=np.float32)
    wt = np.ascontiguousarray(np.asarray(W, dtype=np.float32).T)
    bias = np.ascontiguousarray(np.asarray(b, dtype=np.float32).reshape(1, F))
    return [
        {
            "adj": np.ascontiguousarray(adj_matrix[c]),
            "x": np.ascontiguousarray(node_features[c]),
            "wt": wt,
            "bias": bias,
        }
        for c in range(B)
    ]


def kernel(node_features, adj_matrix, W, b):
    nc = get_nc()
    in_maps = make_in_maps(node_features, adj_matrix, W, b)
    res = run_bass_kernel_spmd(nc, in_maps, core_ids=list(range(B)))
    return np.stack([r["out"] for r in res.results], axis=0)


# revision 4
# speedup vs baseline: 1.1529x; 1.0620x over previous
"""GCN layer kernel for Trainium2 (Bass/Tile), data-parallel over batch.

Reference computation (per batch element):
    deg = A.sum(-1); d = deg ** -0.5
    t   = X @ W.T + b
    out = relu(diag(d) @ A @ diag(d) @ t)

Per-core mapping (8 cores, one batch element each). Host-side staging is
layout/dtype only (transposes + bf16 rounding, same rounding the device
matmul path would apply); all model arithmetic (degree, normalization,
matmuls, bias, relu) runs on device:
  - A is staged twice in bf16: AT (transposed, the matmul stationary) and
    AN (natural, for the on-device degree row-sums). Streaming over the
    contraction index k, AT row-tile k provides the stationary chunks for
    ALL 16 output tiles, so each step runs a uniform batch of 16 products
    (k, mu) — no triangular schedule and no on-device transposes.
  - deg row-sums on DVE (bf16 2x rate) from AN tiles; d = sqrt(1/deg).
  - t = X @ W.T + b in bf16 from host-staged XT/WT; the bias is folded in
    as a K=1 ones x b product that initializes each accumulation group.
    y[k] = d[k] * t[k] rounded to bf16 by ACT.
  - All 16 output chains accumulate in PSUM f32 simultaneously, packed
    2-per-bank across all 8 banks (half-bank sharing: the bank's first
    matmul uses start=True, which marks the whole 2KB zero-region
    pending-zero; the partner chain's first matmul uses start=False and
    overwrites its still-pending half; the bank's last matmul carries
    stop=True). Banks 6/7 first serve the mm1 staging, then host chains
    12..15.
  - Drain: relu(d * psum) on ACT, stores batched 4 row-tiles per
    gpsimd (SWDGE) dma_start.
"""

from contextlib import ExitStack

import numpy as np
import ml_dtypes

import concourse.bacc as bacc
import concourse.mybir as mybir
import concourse.tile as tile
from concourse.bass_utils import run_bass_kernel_spmd

B = 8
N = 2048
F = 256
P = 128
NT = N // P  # 16 row tiles
FT = F // P  # 2 feature chunks
NP_ = NT // 2  # 8 load pairs
F32 = mybir.dt.float32
BF16 = mybir.dt.bfloat16
COPY = mybir.ActivationFunctionType.Copy
RELU = mybir.ActivationFunctionType.Relu
PF_PAIRS = 2  # pairs of (AT, AN) tiles prefetched ahead
STORE_BATCH = 4


def _emit(ctx: ExitStack, tc: tile.TileContext, AT, AN, XT, WTB, BIASB, OUT):
    nc = tc.nc

    const = ctx.enter_context(tc.tile_pool(name="const", bufs=1))
    at_stage = ctx.enter_context(tc.tile_pool(name="at_stage", bufs=3))
    an_stage = ctx.enter_context(tc.tile_pool(name="an_stage", bufs=3))
    scr = ctx.enter_context(tc.tile_pool(name="scr", bufs=2))
    outstage = ctx.enter_context(tc.tile_pool(name="outstage", bufs=2))
    psum_acc = ctx.enter_context(tc.tile_pool(name="psum_acc", bufs=6, space="PSUM"))
    psum_tr = ctx.enter_context(tc.tile_pool(name="psum_tr", bufs=2, space="PSUM"))

    # ---- head DMA: small bf16 operands, then the paired A streams ----
    wt_sb = const.tile([P, FT * F], BF16, tag="wt")
    nc.sync.dma_start(
        out=wt_sb[:, :].rearrange("p (c f) -> p c f", c=FT),
        in_=WTB.rearrange("(c p) f -> p c f", p=P),
    )
    b_bf = const.tile([1, F], BF16, tag="bbf")
    nc.sync.dma_start(out=b_bf[:, :], in_=BIASB[:, :])
    xt_sb = const.tile([P, FT * N], BF16, tag="xt")
    nc.sync.dma_start(
        out=xt_sb[:, :].rearrange("p (c n) -> p c n", c=FT),
        in_=XT.rearrange("(c p) n -> p c n", p=P),
    )

    at_tiles = {}
    an_tiles = {}

    def emit_load_pair(pr):
        at_tiles[pr] = at_stage.tile([P, 2 * N], BF16, tag="at", name=f"at_{pr}")
        nc.sync.dma_start(
            out=at_tiles[pr][:, :].rearrange("p (t n) -> p t n", t=2),
            in_=AT.rearrange("(t p) n -> p t n", p=P)[:, 2 * pr : 2 * pr + 2, :],
        )
        an_tiles[pr] = an_stage.tile([P, 2 * N], BF16, tag="an", name=f"an_{pr}")
        nc.sync.dma_start(
            out=an_tiles[pr][:, :].rearrange("p (t n) -> p t n", t=2),
            in_=AN.rearrange("(t p) n -> p t n", p=P)[:, 2 * pr : 2 * pr + 2, :],
        )

    for pr in range(PF_PAIRS):
        emit_load_pair(pr)

    ones_bf = const.tile([1, P], BF16, tag="ones")
    nc.vector.memset(ones_bf[:, :], 1.0)

    deg = const.tile([P, NT], F32, tag="deg")
    rec = const.tile([P, NT], F32, tag="rec")
    dinv = const.tile([P, NT], F32, tag="dinv")
    t_big = const.tile([P, NT * F], F32, tag="t")
    y_big = const.tile([P, NT * F], BF16, tag="y")

    # ---- mm1: t[j] = X @ W.T + b, bf16, through the tr-bank rotation ----
    for j in range(NT):
        t_psum = psum_tr.tile([P, F], F32, tag="tr", name=f"t_psum_{j}")
        nc.tensor.matmul(t_psum[:, :], ones_bf[:, :], b_bf[:, :], start=True, stop=False)
        for phi in range(FT):
            nc.tensor.matmul(
                t_psum[:, :],
                xt_sb[:, phi * N + j * P : phi * N + (j + 1) * P],
                wt_sb[:, phi * F : (phi + 1) * F],
                start=False,
                stop=(phi == FT - 1),
            )
        nc.scalar.copy(t_big[:, j * F : (j + 1) * F], t_psum[:, :])

    # ---- all 16 accumulation chains, 2 per bank ----
    acc_banks = [
        psum_acc.tile([P, 2 * F], F32, tag="acc", name=f"accbank_{b_}")
        for b_ in range(6)
    ]
    cbank_a = psum_tr.tile([P, 2 * F], F32, tag="tr", name="cbank_a")  # chains 12,13
    cbank_b = psum_tr.tile([P, 2 * F], F32, tag="tr", name="cbank_b")  # chains 14,15

    def acc_region(mu):
        half = (mu % 2) * F
        if mu < 12:
            return acc_banks[mu // 2][:, half : half + F]
        return (cbank_a if mu < 14 else cbank_b)[:, half : half + F]

    ostiles = {}

    def emit_drain(mu):
        bi = mu // STORE_BATCH
        if bi not in ostiles:
            ostiles[bi] = outstage.tile(
                [P, STORE_BATCH * F], F32, tag="os", name=f"os_{bi}"
            )
        j = mu % STORE_BATCH
        nc.scalar.activation(
            ostiles[bi][:, j * F : (j + 1) * F], acc_region(mu), RELU,
            scale=dinv[:, mu : mu + 1],
        )
        if j == STORE_BATCH - 1:
            lo = bi * STORE_BATCH
            nc.gpsimd.dma_start(
                out=OUT.rearrange("(m p) f -> p m f", p=P)[:, lo : lo + STORE_BATCH, :],
                in_=ostiles[bi][:, :].rearrange("p (m f) -> p m f", m=STORE_BATCH),
            )

    # ---- stream over the contraction index k ----
    for pr in range(NP_):
        if pr + PF_PAIRS < NP_:
            emit_load_pair(pr + PF_PAIRS)
        at_pair = at_tiles.pop(pr)
        an_pair = an_tiles.pop(pr)
        for h in range(2):
            k = 2 * pr + h
            # degree row-sums (bf16 2x rate), d = sqrt(1/deg)
            sc = scr.tile([P, N], BF16, tag="sc", name=f"sc_{k}")
            nc.vector.tensor_scalar(
                out=sc[:, :],
                in0=an_pair[:, h * N : (h + 1) * N],
                scalar1=0.0,
                scalar2=None,
                op0=mybir.AluOpType.add,
                op1=mybir.AluOpType.add,
                accum_out=deg[:, k : k + 1],
            )
            nc.vector.reciprocal(rec[:, k : k + 1], deg[:, k : k + 1])
            nc.scalar.sqrt(dinv[:, k : k + 1], rec[:, k : k + 1])
            nc.scalar.activation(
                y_big[:, k * F : (k + 1) * F],
                t_big[:, k * F : (k + 1) * F],
                COPY,
                scale=dinv[:, k : k + 1],
            )
            # one uniform batch of products: every output tile consumes y[k]
            for mu in range(NT):
                nc.tensor.matmul(
                    acc_region(mu),
                    at_pair[:, h * N + mu * P : h * N + (mu + 1) * P],
                    y_big[:, k * F : (k + 1) * F],
                    start=(k == 0 and mu % 2 == 0),
                    stop=(k == NT - 1 and mu % 2 == 1),
                )

    # ---- tail: relu(d * acc) and batched stores ----
    for mu in range(NT):
        emit_drain(mu)


_cached_nc = None


def _build():
    nc = bacc.Bacc("TRN2", target_bir_lowering=False, debug=False)
    AT = nc.dram_tensor("at", [N, N], BF16, kind="ExternalInput").ap()
    AN = nc.dram_tensor("an", [N, N], BF16, kind="ExternalInput").ap()
    XT = nc.dram_tensor("xt", [F, N], BF16, kind="ExternalInput").ap()
    WTB = nc.dram_tensor("wtb", [F, F], BF16, kind="ExternalInput").ap()
    BIASB = nc.dram_tensor("biasb", [1, F], BF16, kind="ExternalInput").ap()
    OUT = nc.dram_tensor("out", [N, F], F32, kind="ExternalOutput").ap()
    with tile.TileContext(nc) as tc:
        with ExitStack() as ctx:
            _emit(ctx, tc, AT, AN, XT, WTB, BIASB, OUT)
    nc.compile()
    return nc


def get_nc():
    global _cached_nc
    if _cached_nc is None:
        _cached_nc = _build()
    return _cached_nc


def make_in_maps(node_features, adj_matrix, W, b):
    bf16 = ml_dtypes.bfloat16
    node_features = np.asarray(node_features, dtype=np.float32)
    adj_matrix = np.asarray(adj_matrix, dtype=np.float32)
    an = adj_matrix.astype(bf16)  # [B, N, N] natural
    at = np.ascontiguousarray(an.transpose(0, 2, 1))  # [B, N, N] transposed
    xt = np.ascontiguousarray(
        node_features.astype(bf16).transpose(0, 2, 1)
    )  # [B, F, N]
    wtb = np.ascontiguousarray(np.asarray(W, dtype=np.float32).T.astype(bf16))
    biasb = np.ascontiguousarray(
        np.asarray(b, dtype=np.float32).reshape(1, F).astype(bf16)
    )
    return [
        {
            "at": np.ascontiguousarray(at[c]),
            "an": np.ascontiguousarray(an[c]),
            "xt": xt[c],
            "wtb": wtb,
            "biasb": biasb,
        }
        for c in range(B)
    ]


def kernel(node_features, adj_matrix, W, b):
    nc = get_nc()
    in_maps = make_in_maps(node_features, adj_matrix, W, b)
    res = run_bass_kernel_spmd(nc, in_maps, core_ids=list(range(B)))
    return np.stack([r["out"] for r in res.results], axis=0)


# revision 7
# speedup vs baseline: 1.2815x; 1.1115x over previous
"""GCN layer kernel for Trainium2 (Bass/Tile), data-parallel over batch.

Reference computation (per batch element):
    deg = A.sum(-1); d = deg ** -0.5
    t   = X @ W.T + b
    out = relu(diag(d) @ A @ diag(d) @ t)

Per-core mapping (8 cores, one batch element each). Host-side staging is
layout/dtype only (transposes + bf16 rounding, same rounding the device
matmul path would apply); all model arithmetic (degree, normalization,
matmuls, bias, relu) runs on device:
  - A is staged twice in bf16: AT (transposed, the matmul stationary) and
    AN (natural, for the on-device degree row-sums). Streaming over the
    contraction index k, AT row-tile k provides the stationary chunks for
    ALL 16 output tiles, so each step runs a uniform batch of 16 products
    (k, mu) — no triangular schedule and no on-device transposes.
  - deg row-sums on DVE (bf16 2x rate) from AN tiles; d = sqrt(1/deg).
  - t = X @ W.T + b in bf16 from host-staged XT/WT; the bias is folded in
    as a K=1 ones x b product that initializes each accumulation group.
    y[k] = d[k] * t[k] rounded to bf16 by ACT.
  - All 16 output chains accumulate in PSUM f32 simultaneously, packed
    2-per-bank across all 8 banks (half-bank sharing: the bank's first
    matmul uses start=True, which marks the whole 2KB zero-region
    pending-zero; the partner chain's first matmul uses start=False and
    overwrites its still-pending half; the bank's last matmul carries
    stop=True). Banks 6/7 first serve the mm1 staging, then host chains
    12..15.
  - Drain: relu(d * psum) on ACT, stores batched 4 row-tiles per
    gpsimd (SWDGE) dma_start.
"""

from contextlib import ExitStack

import numpy as np
import ml_dtypes

import concourse.bacc as bacc
import concourse.mybir as mybir
import concourse.tile as tile
from concourse.bass_utils import run_bass_kernel_spmd

B = 8
N = 2048
F = 256
P = 128
NT = N // P  # 16 row tiles
FT = F // P  # 2 feature chunks
NP_ = NT // 2  # 8 load pairs
F32 = mybir.dt.float32
BF16 = mybir.dt.bfloat16
COPY = mybir.ActivationFunctionType.Copy
RELU = mybir.ActivationFunctionType.Relu
PF_PAIRS = 3  # pairs of (AT, AN) tiles prefetched ahead
STORE_BATCH = 4


def _emit(ctx: ExitStack, tc: tile.TileContext, AT, AN, XT, WTB, BIASB, OUT):
    nc = tc.nc

    const = ctx.enter_context(tc.tile_pool(name="const", bufs=1))
    at_stage = ctx.enter_context(tc.tile_pool(name="at_stage", bufs=4))
    an_stage = ctx.enter_context(tc.tile_pool(name="an_stage", bufs=4))
    scr = ctx.enter_context(tc.tile_pool(name="scr", bufs=3))
    outstage = ctx.enter_context(tc.tile_pool(name="outstage", bufs=2))
    psum_acc = ctx.enter_context(tc.tile_pool(name="psum_acc", bufs=6, space="PSUM"))
    psum_tr = ctx.enter_context(tc.tile_pool(name="psum_tr", bufs=2, space="PSUM"))

    # ---- head DMA: small bf16 operands, then the paired A streams ----
    wt_sb = const.tile([P, FT * F], BF16, tag="wt")
    nc.sync.dma_start(
        out=wt_sb[:, :].rearrange("p (c f) -> p c f", c=FT),
        in_=WTB.rearrange("(c p) f -> p c f", p=P),
    )
    b_bf = const.tile([1, F], BF16, tag="bbf")
    nc.sync.dma_start(out=b_bf[:, :], in_=BIASB[:, :])
    xt_sb = const.tile([P, FT * N], BF16, tag="xt")
    nc.sync.dma_start(
        out=xt_sb[:, :].rearrange("p (c n) -> p c n", c=FT),
        in_=XT.rearrange("(c p) n -> p c n", p=P),
    )

    at_tiles = {}
    an_tiles = {}

    def emit_load_pair(pr):
        at_tiles[pr] = at_stage.tile([P, 2 * N], BF16, tag="at", name=f"at_{pr}")
        nc.sync.dma_start(
            out=at_tiles[pr][:, :].rearrange("p (t n) -> p t n", t=2),
            in_=AT.rearrange("(t p) n -> p t n", p=P)[:, 2 * pr : 2 * pr + 2, :],
        )
        an_tiles[pr] = an_stage.tile([P, 2 * N], BF16, tag="an", name=f"an_{pr}")
        nc.sync.dma_start(
            out=an_tiles[pr][:, :].rearrange("p (t n) -> p t n", t=2),
            in_=AN.rearrange("(t p) n -> p t n", p=P)[:, 2 * pr : 2 * pr + 2, :],
        )

    for pr in range(PF_PAIRS):
        emit_load_pair(pr)

    ones_bf = const.tile([1, P], BF16, tag="ones")
    nc.vector.memset(ones_bf[:, :], 1.0)

    deg = const.tile([P, NT], F32, tag="deg")
    rec = const.tile([P, NT], F32, tag="rec")
    dinv = const.tile([P, NT], F32, tag="dinv")
    t_big = const.tile([P, NT * F], F32, tag="t")
    y_big = const.tile([P, NT * F], BF16, tag="y")

    # ---- mm1: t[j] = X @ W.T + b, bf16, through the tr-bank rotation ----
    for j in range(NT):
        t_psum = psum_tr.tile([P, F], F32, tag="tr", name=f"t_psum_{j}")
        nc.tensor.matmul(t_psum[:, :], ones_bf[:, :], b_bf[:, :], start=True, stop=False)
        for phi in range(FT):
            nc.tensor.matmul(
                t_psum[:, :],
                xt_sb[:, phi * N + j * P : phi * N + (j + 1) * P],
                wt_sb[:, phi * F : (phi + 1) * F],
                start=False,
                stop=(phi == FT - 1),
            )
        nc.scalar.copy(t_big[:, j * F : (j + 1) * F], t_psum[:, :])

    # ---- all 16 accumulation chains, 2 per bank ----
    acc_banks = [
        psum_acc.tile([P, 2 * F], F32, tag="acc", name=f"accbank_{b_}")
        for b_ in range(6)
    ]
    cbank_a = psum_tr.tile([P, 2 * F], F32, tag="tr", name="cbank_a")  # chains 12,13
    cbank_b = psum_tr.tile([P, 2 * F], F32, tag="tr", name="cbank_b")  # chains 14,15

    def acc_region(mu):
        half = (mu % 2) * F
        if mu < 12:
            return acc_banks[mu // 2][:, half : half + F]
        return (cbank_a if mu < 14 else cbank_b)[:, half : half + F]

    ostiles = {}

    def emit_drain(mu):
        bi = mu // STORE_BATCH
        if bi not in ostiles:
            ostiles[bi] = outstage.tile(
                [P, STORE_BATCH * F], F32, tag="os", name=f"os_{bi}"
            )
        j = mu % STORE_BATCH
        nc.scalar.activation(
            ostiles[bi][:, j * F : (j + 1) * F], acc_region(mu), RELU,
            scale=dinv[:, mu : mu + 1],
        )
        if j == STORE_BATCH - 1:
            lo = bi * STORE_BATCH
            nc.gpsimd.dma_start(
                out=OUT.rearrange("(m p) f -> p m f", p=P)[:, lo : lo + STORE_BATCH, :],
                in_=ostiles[bi][:, :].rearrange("p (m f) -> p m f", m=STORE_BATCH),
            )

    # ---- stream over the contraction index k ----
    for pr in range(NP_):
        if pr + PF_PAIRS < NP_:
            emit_load_pair(pr + PF_PAIRS)
        at_pair = at_tiles.pop(pr)
        an_pair = an_tiles.pop(pr)
        for h in range(2):
            k = 2 * pr + h
            # degree row-sums, split across DVE (even k) and ACT (odd k)
            sc = scr.tile([P, N], BF16, tag="sc", name=f"sc_{k}")
            if k % 2 == 0:
                nc.vector.tensor_scalar(
                    out=sc[:, :],
                    in0=an_pair[:, h * N : (h + 1) * N],
                    scalar1=0.0,
                    scalar2=None,
                    op0=mybir.AluOpType.add,
                    op1=mybir.AluOpType.add,
                    accum_out=deg[:, k : k + 1],
                )
            else:
                nc.scalar.activation(
                    sc[:, :],
                    an_pair[:, h * N : (h + 1) * N],
                    COPY,
                    accum_out=deg[:, k : k + 1],
                )
            nc.vector.reciprocal(rec[:, k : k + 1], deg[:, k : k + 1])
            nc.scalar.sqrt(dinv[:, k : k + 1], rec[:, k : k + 1])
            nc.scalar.activation(
                y_big[:, k * F : (k + 1) * F],
                t_big[:, k * F : (k + 1) * F],
                COPY,
                scale=dinv[:, k : k + 1],
            )
            # one uniform batch of products: every output tile consumes y[k]
            for mu in range(NT):
                nc.tensor.matmul(
                    acc_region(mu),
                    at_pair[:, h * N + mu * P : h * N + (mu + 1) * P],
                    y_big[:, k * F : (k + 1) * F],
                    start=(k == 0 and mu % 2 == 0),
                    stop=(k == NT - 1 and mu % 2 == 1),
                )

    # ---- tail: relu(d * acc) and batched stores ----
    for mu in range(NT):
        emit_drain(mu)


_cached_nc = None


def _build():
    nc = bacc.Bacc("TRN2", target_bir_lowering=False, debug=False)
    AT = nc.dram_tensor("at", [N, N], BF16, kind="ExternalInput").ap()
    AN = nc.dram_tensor("an", [N, N], BF16, kind="ExternalInput").ap()
    XT = nc.dram_tensor("xt", [F, N], BF16, kind="ExternalInput").ap()
    WTB = nc.dram_tensor("wtb", [F, F], BF16, kind="ExternalInput").ap()
    BIASB = nc.dram_tensor("biasb", [1, F], BF16, kind="ExternalInput").ap()
    OUT = nc.dram_tensor("out", [N, F], F32, kind="ExternalOutput").ap()
    with tile.TileContext(nc) as tc:
        with ExitStack() as ctx:
            _emit(ctx, tc, AT, AN, XT, WTB, BIASB, OUT)
    nc.compile()
    return nc


def get_nc():
    global _cached_nc
    if _cached_nc is None:
        _cached_nc = _build()
    return _cached_nc


def make_in_maps(node_features, adj_matrix, W, b):
    bf16 = ml_dtypes.bfloat16
    node_features = np.asarray(node_features, dtype=np.float32)
    adj_matrix = np.asarray(adj_matrix, dtype=np.float32)
    an = adj_matrix.astype(bf16)  # [B, N, N] natural
    at = np.ascontiguousarray(an.transpose(0, 2, 1))  # [B, N, N] transposed
    xt = np.ascontiguousarray(
        node_features.astype(bf16).transpose(0, 2, 1)
    )  # [B, F, N]
    wtb = np.ascontiguousarray(np.asarray(W, dtype=np.float32).T.astype(bf16))
    biasb = np.ascontiguousarray(
        np.asarray(b, dtype=np.float32).reshape(1, F).astype(bf16)
    )
    return [
        {
            "at": np.ascontiguousarray(at[c]),
            "an": np.ascontiguousarray(an[c]),
            "xt": xt[c],
            "wtb": wtb,
            "biasb": biasb,
        }
        for c in range(B)
    ]


def kernel(node_features, adj_matrix, W, b):
    nc = get_nc()
    in_maps = make_in_maps(node_features, adj_matrix, W, b)
    res = run_bass_kernel_spmd(nc, in_maps, core_ids=list(range(B)))
    return np.stack([r["out"] for r in res.results], axis=0)


# revision 12
# speedup vs baseline: 1.3380x; 1.0441x over previous
"""GCN layer kernel for Trainium2 (Bass/Tile), data-parallel over batch.

Reference computation (per batch element):
    deg = A.sum(-1); d = deg ** -0.5
    t   = X @ W.T + b
    out = relu(diag(d) @ A @ diag(d) @ t)

Per-core mapping (8 cores, one batch element each). Host-side staging is
layout/dtype only (transposes + bf16 rounding, same rounding the device
matmul path would apply); all model arithmetic (degree, normalization,
matmuls, bias, relu) runs on device:
  - A is staged twice in bf16: AT (transposed, the matmul stationary) and
    AN (natural, for the on-device degree row-sums). Streaming over the
    contraction index k, AT row-tile k provides the stationary chunks for
    ALL 16 output tiles, so each step runs a uniform batch of 16 products
    (k, mu) — no triangular schedule and no on-device transposes.
  - deg row-sums on DVE (bf16 2x rate) from AN tiles; d = sqrt(1/deg).
  - t = X @ W.T + b in bf16 from host-staged XT/WT; the bias is folded in
    as a K=1 ones x b product that initializes each accumulation group.
    y[k] = d[k] * t[k] rounded to bf16 by ACT.
  - All 16 output chains accumulate in PSUM f32 simultaneously, packed
    2-per-bank across all 8 banks (half-bank sharing: the bank's first
    matmul uses start=True, which marks the whole 2KB zero-region
    pending-zero; the partner chain's first matmul uses start=False and
    overwrites its still-pending half; the bank's last matmul carries
    stop=True). Banks 6/7 first serve the mm1 staging, then host chains
    12..15.
  - Drain: relu(d * psum) on ACT, stores batched 4 row-tiles per
    gpsimd (SWDGE) dma_start.
"""

from contextlib import ExitStack

import numpy as np
import ml_dtypes

import concourse.bacc as bacc
import concourse.mybir as mybir
import concourse.tile as tile
from concourse.bass_utils import run_bass_kernel_spmd

B = 8
N = 2048
F = 256
P = 128
NT = N // P  # 16 row tiles
FT = F // P  # 2 feature chunks
NP_ = NT // 2  # 8 load pairs
F32 = mybir.dt.float32
BF16 = mybir.dt.bfloat16
COPY = mybir.ActivationFunctionType.Copy
RELU = mybir.ActivationFunctionType.Relu
PF_PAIRS = 3  # pairs of (AT, AN) tiles prefetched ahead
STORE_BATCH = 4


def _emit(ctx: ExitStack, tc: tile.TileContext, AT, AN, XT, WTB, BIASB, OUT):
    nc = tc.nc

    const = ctx.enter_context(tc.tile_pool(name="const", bufs=1))
    at_stage = ctx.enter_context(tc.tile_pool(name="at_stage", bufs=5))
    an_stage = ctx.enter_context(tc.tile_pool(name="an_stage", bufs=5))
    scr = ctx.enter_context(tc.tile_pool(name="scr", bufs=3))
    outstage = ctx.enter_context(tc.tile_pool(name="outstage", bufs=2))
    psum_acc = ctx.enter_context(tc.tile_pool(name="psum_acc", bufs=6, space="PSUM"))
    psum_tr = ctx.enter_context(tc.tile_pool(name="psum_tr", bufs=2, space="PSUM"))

    # ---- head DMA: first A pair leads, small bf16 operands slot behind ----
    at_tiles = {}
    an_tiles = {}

    def emit_load_pair(pr):
        an_tiles[pr] = an_stage.tile([P, 2 * N], BF16, tag="an", name=f"an_{pr}")
        nc.sync.dma_start(
            out=an_tiles[pr][:, :].rearrange("p (t n) -> p t n", t=2),
            in_=AN.rearrange("(t p) n -> p t n", p=P)[:, 2 * pr : 2 * pr + 2, :],
        )
        at_tiles[pr] = at_stage.tile([P, 2 * N], BF16, tag="at", name=f"at_{pr}")
        nc.sync.dma_start(
            out=at_tiles[pr][:, :].rearrange("p (t n) -> p t n", t=2),
            in_=AT.rearrange("(t p) n -> p t n", p=P)[:, 2 * pr : 2 * pr + 2, :],
        )

    emit_load_pair(0)
    wt_sb = const.tile([P, FT * F], BF16, tag="wt")
    nc.sync.dma_start(
        out=wt_sb[:, :].rearrange("p (c f) -> p c f", c=FT),
        in_=WTB.rearrange("(c p) f -> p c f", p=P),
    )
    b_bf = const.tile([1, F], BF16, tag="bbf")
    nc.sync.dma_start(out=b_bf[:, :], in_=BIASB[:, :])
    xt_sb = const.tile([P, FT * N], BF16, tag="xt")
    nc.sync.dma_start(
        out=xt_sb[:, :].rearrange("p (c n) -> p c n", c=FT),
        in_=XT.rearrange("(c p) n -> p c n", p=P),
    )
    for pr in range(1, PF_PAIRS):
        emit_load_pair(pr)

    ones_bf = const.tile([1, P], BF16, tag="ones")
    nc.vector.memset(ones_bf[:, :], 1.0)

    deg = const.tile([P, NT], F32, tag="deg")
    rec = const.tile([P, NT], F32, tag="rec")
    dinv = const.tile([P, NT], F32, tag="dinv")
    t_big = const.tile([P, NT * F], F32, tag="t")
    y_big = const.tile([P, NT * F], BF16, tag="y")

    # ---- mm1: t[j] = X @ W.T + b, bf16, through the tr-bank rotation ----
    for j in range(NT):
        t_psum = psum_tr.tile([P, F], F32, tag="tr", name=f"t_psum_{j}")
        nc.tensor.matmul(t_psum[:, :], ones_bf[:, :], b_bf[:, :], start=True, stop=False)
        for phi in range(FT):
            nc.tensor.matmul(
                t_psum[:, :],
                xt_sb[:, phi * N + j * P : phi * N + (j + 1) * P],
                wt_sb[:, phi * F : (phi + 1) * F],
                start=False,
                stop=(phi == FT - 1),
            )
        nc.scalar.copy(t_big[:, j * F : (j + 1) * F], t_psum[:, :])

    # ---- all 16 accumulation chains, 2 per bank ----
    acc_banks = [
        psum_acc.tile([P, 2 * F], F32, tag="acc", name=f"accbank_{b_}")
        for b_ in range(6)
    ]
    cbank_a = psum_tr.tile([P, 2 * F], F32, tag="tr", name="cbank_a")  # chains 12,13
    cbank_b = psum_tr.tile([P, 2 * F], F32, tag="tr", name="cbank_b")  # chains 14,15

    def acc_region(mu):
        half = (mu % 2) * F
        if mu < 12:
            return acc_banks[mu // 2][:, half : half + F]
        return (cbank_a if mu < 14 else cbank_b)[:, half : half + F]

    ostiles = {}

    def emit_drain(mu):
        bi = mu // STORE_BATCH
        if bi not in ostiles:
            ostiles[bi] = outstage.tile(
                [P, STORE_BATCH * F], F32, tag="os", name=f"os_{bi}"
            )
        j = mu % STORE_BATCH
        dst = ostiles[bi][:, j * F : (j + 1) * F]
        if mu % 2 == 0:
            # relu(d * psum) on ACT
            nc.scalar.activation(
                dst, acc_region(mu), RELU, scale=dinv[:, mu : mu + 1]
            )
        else:
            # same on DVE: (psum * d) max 0
            nc.vector.tensor_scalar(
                out=dst,
                in0=acc_region(mu),
                scalar1=dinv[:, mu : mu + 1],
                scalar2=0.0,
                op0=mybir.AluOpType.mult,
                op1=mybir.AluOpType.max,
            )
        if j == STORE_BATCH - 1:
            lo = bi * STORE_BATCH
            q = nc.sync if (bi % 2 == 0) else nc.gpsimd
            q.dma_start(
                out=OUT.rearrange("(m p) f -> p m f", p=P)[:, lo : lo + STORE_BATCH, :],
                in_=ostiles[bi][:, :].rearrange("p (m f) -> p m f", m=STORE_BATCH),
            )

    # ---- stream over the contraction index k ----
    for pr in range(NP_):
        if pr + PF_PAIRS < NP_:
            emit_load_pair(pr + PF_PAIRS)
        at_pair = at_tiles.pop(pr)
        an_pair = an_tiles.pop(pr)
        for h in range(2):
            k = 2 * pr + h
            # degree row-sums, split across DVE (even k) and ACT (odd k)
            sc = scr.tile([P, N], BF16, tag="sc", name=f"sc_{k}")
            if k % 2 == 0:
                nc.vector.tensor_scalar(
                    out=sc[:, :],
                    in0=an_pair[:, h * N : (h + 1) * N],
                    scalar1=0.0,
                    scalar2=None,
                    op0=mybir.AluOpType.add,
                    op1=mybir.AluOpType.add,
                    accum_out=deg[:, k : k + 1],
                )
            else:
                nc.scalar.activation(
                    sc[:, :],
                    an_pair[:, h * N : (h + 1) * N],
                    COPY,
                    accum_out=deg[:, k : k + 1],
                )
            nc.vector.reciprocal(rec[:, k : k + 1], deg[:, k : k + 1])
            nc.scalar.sqrt(dinv[:, k : k + 1], rec[:, k : k + 1])
            nc.scalar.activation(
                y_big[:, k * F : (k + 1) * F],
                t_big[:, k * F : (k + 1) * F],
                COPY,
                scale=dinv[:, k : k + 1],
            )
            # one uniform batch of products: every output tile consumes y[k]
            for mu in range(NT):
                nc.tensor.matmul(
                    acc_region(mu),
                    at_pair[:, h * N + mu * P : h * N + (mu + 1) * P],
                    y_big[:, k * F : (k + 1) * F],
                    start=(k == 0 and mu % 2 == 0),
                    stop=(k == NT - 1 and mu % 2 == 1),
                )

    # ---- tail: relu(d * acc) and batched stores ----
    for mu in range(NT):
        emit_drain(mu)


_cached_nc = None


def _build():
    nc = bacc.Bacc("TRN2", target_bir_lowering=False, debug=False)
    AT = nc.dram_tensor("at", [N, N], BF16, kind="ExternalInput").ap()
    AN = nc.dram_tensor("an", [N, N], BF16, kind="ExternalInput").ap()
    XT = nc.dram_tensor("xt", [F, N], BF16, kind="ExternalInput").ap()
    WTB = nc.dram_tensor("wtb", [F, F], BF16, kind="ExternalInput").ap()
    BIASB = nc.dram_tensor("biasb", [1, F], BF16, kind="ExternalInput").ap()
    OUT = nc.dram_tensor("out", [N, F], F32, kind="ExternalOutput").ap()
    with tile.TileContext(nc) as tc:
        with ExitStack() as ctx:
            _emit(ctx, tc, AT, AN, XT, WTB, BIASB, OUT)
    nc.compile()
    return nc


def get_nc():
    global _cached_nc
    if _cached_nc is None:
        _cached_nc = _build()
    return _cached_nc


def make_in_maps(node_features, adj_matrix, W, b):
    bf16 = ml_dtypes.bfloat16
    node_features = np.asarray(node_features, dtype=np.float32)
    adj_matrix = np.asarray(adj_matrix, dtype=np.float32)
    an = adj_matrix.astype(bf16)  # [B, N, N] natural
    at = np.ascontiguousarray(an.transpose(0, 2, 1))  # [B, N, N] transposed
    xt = np.ascontiguousarray(
        node_features.astype(bf16).transpose(0, 2, 1)
    )  # [B, F, N]
    wtb = np.ascontiguousarray(np.asarray(W, dtype=np.float32).T.astype(bf16))
    biasb = np.ascontiguousarray(
        np.asarray(b, dtype=np.float32).reshape(1, F).astype(bf16)
    )
    return [
        {
            "at": np.ascontiguousarray(at[c]),
            "an": np.ascontiguousarray(an[c]),
            "xt": xt[c],
            "wtb": wtb,
            "biasb": biasb,
        }
        for c in range(B)
    ]


def kernel(node_features, adj_matrix, W, b):
    nc = get_nc()
    in_maps = make_in_maps(node_features, adj_matrix, W, b)
    res = run_bass_kernel_spmd(nc, in_maps, core_ids=list(range(B)))
    return np.stack([r["out"] for r in res.results], axis=0)
